# revision 8
# baseline (speedup 1.0000x reference)
"""CharLevelEncoder Trainium2 kernel (8-core SPMD).

Math: out = relu(concat(word_emb[word_ids], h(char_ids)) @ W_lin.T + b_lin)
with h a single LSTM cell step from zero state on E[char_ids].

Key algebraic restructuring:
  - h depends only on char_id (40 values) -> HB table [40, WD]:
        HB[c] = h_c @ W_lin[:, WD:].T + b_lin
  - word_emb[word_ids] @ W_lin[:, :WD].T == (word_emb @ A.T)[word_ids]
    (A = W_lin[:, :WD]); P computed per used word, sharded by word bins.
  - out[t] = relu(P[word_ids[t]] + HB[char_ids[t]])

Implementation notes (v2, measured-cost-model driven):
  - matmul cost on TRN2 ~= rhs-free-columns x 0.42ns regardless of
    contraction rows; fp8 DoubleRow gives no net win for 16-bit-precision
    data (2x8-bit slots/cycle == 1x16-bit), so all matmul data is fp16
    (same speed as bf16, 4 extra mantissa bits).
  - N=1024 matmuls (PSUM out spans 2 banks) halve instruction count vs
    N=512: 8 P-GEMM matmuls + 2 expansion matmuls per tile.
  - one-hot operands (sel slot-gather, oc char-id HB-add) are fp8e4m3
    (exact for 0/1, halves their DMA).
  - output stored fp16 (halves the dominant DMA stream vs f32).
  - software pipeline: iter c runs P-copy(c) on ACT/DVE, P-GEMM(c+1) on
    PE, then expansion(c) on PE, so the PE never waits on the copies.
"""

import os

import ml_dtypes
import numpy as np

import concourse.bass as bass
import concourse.tile as tile
from concourse import bacc, mybir
from concourse.bass_utils import run_bass_kernel_spmd

NCORES = 8
WD = 1024
NE = 40
HID = 512
BIN_CHAR_CAP = 512
BIN_WORD_CAP = 128

# N=1024 two-bank PSUM matmuls are rejected by hardware (PSUM bank
# boundary), so N=512 is the only live path.
N1024 = os.environ.get("KERNEL_N1024", "0") == "1"

NP16 = np.float16
NP8 = ml_dtypes.float8_e4m3


def _sigmoid(x):
    return 1.0 / (1.0 + np.exp(-x))


def _hb_table(E, W_ih, b_ih, b_hh, W_lin, b_lin):
    G = E.astype(np.float32) @ W_ih.T + b_ih + b_hh  # [NE, 4H]
    i, f, g, o = np.split(G, 4, axis=1)
    c = _sigmoid(i) * np.tanh(g)
    h = _sigmoid(o) * np.tanh(c)  # [NE, H]
    return (h @ W_lin[:, WD:].T + b_lin).astype(np.float32)  # [NE, WD]


def _pack_bins(word_ids, n_words):
    """Pack words into bins of <=BIN_WORD_CAP words and <=BIN_CHAR_CAP chars.
    Snake-deal of words sorted by char count keeps bin char totals equal."""
    wc = np.bincount(word_ids, minlength=n_words)
    used = np.nonzero(wc)[0]
    counts = wc[used]
    assert counts.max() <= BIN_CHAR_CAP, "single word exceeds bin capacity"
    order = np.argsort(-counts, kind="stable")
    wsorted = used[order]
    csorted = counts[order]
    nused = len(used)
    total = int(counts.sum())

    nbins = max(int(np.ceil(nused / (BIN_WORD_CAP - 2))), int(np.ceil(total / (BIN_CHAR_CAP - 12))))
    nbins = ((nbins + NCORES - 1) // NCORES) * NCORES
    while True:
        bin_of = np.empty(nused, np.int32)
        for r in range(int(np.ceil(nused / nbins))):
            lo, hi = r * nbins, min((r + 1) * nbins, nused)
            idx = np.arange(lo, hi)
            if r % 2 == 0:
                bin_of[idx] = idx - lo
            else:
                bin_of[idx] = nbins - 1 - (idx - lo)
        sums = np.bincount(bin_of, weights=csorted, minlength=nbins)
        nword = np.bincount(bin_of, minlength=nbins)
        if sums.max() <= BIN_CHAR_CAP and nword.max() <= BIN_WORD_CAP:
            break
        nbins += NCORES
    # Repair pass: concentrate the slack into the NCORES*4 smallest bins so
    # most chunks fill whole 128-char tiles (fewer tiles => less padding).
    import bisect
    ndon = min(4 * NCORES, nbins)
    order_bins = np.argsort(-sums, kind="stable")
    donors = set(order_bins[nbins - ndon:].tolist())
    pool = sorted((int(csorted[j]), j) for j in range(nused)
                  if int(bin_of[j]) in donors)  # asc by count
    keys = [c for c, _ in pool]
    for b in order_bins[:nbins - ndon]:
        deficit = int(BIN_CHAR_CAP - sums[b])
        while deficit > 0 and nword[b] < BIN_WORD_CAP and pool:
            k = bisect.bisect_right(keys, deficit) - 1
            if k < 0:
                break
            c, j = pool.pop(k)
            keys.pop(k)
            src = int(bin_of[j])
            bin_of[j] = b
            sums[b] += c
            sums[src] -= c
            nword[b] += 1
            nword[src] -= 1
            deficit -= c

    word_bin = np.full(n_words, -1, np.int32)
    word_bin[wsorted] = bin_of
    word_slot = np.full(n_words, -1, np.int32)
    ord2 = np.argsort(bin_of, kind="stable")
    slots = np.arange(nused) - np.concatenate([[0], np.cumsum(np.bincount(bin_of, minlength=nbins))])[bin_of[ord2]]
    word_slot[wsorted[ord2]] = slots
    return word_bin, word_slot, nbins, sums, nword


def _build_program(nch, tiles_per_chunk):
    ntiles = int(np.sum(tiles_per_chunk))
    f32 = mybir.dt.float32
    f16 = mybir.dt.float16
    fp8 = mybir.dt.float8e4
    nc = bacc.Bacc("TRN2", target_bir_lowering=False, debug=False, num_devices=NCORES)
    weTb_ap = nc.dram_tensor("weTb", [nch, 128, WD], f16, kind="ExternalInput").ap()
    ATb_ap = nc.dram_tensor("ATb", [8, 128, WD], f16, kind="ExternalInput").ap()
    HBp_ap = nc.dram_tensor("HBp", [NE, WD], f16, kind="ExternalInput").ap()
    sel_ap = nc.dram_tensor("sel", [128, ntiles * 128], fp8, kind="ExternalInput").ap()
    ocid_ap = nc.dram_tensor("ocid", [NE, ntiles * 128], fp8, kind="ExternalInput").ap()
    out_ap = nc.dram_tensor("out", [ntiles * 128, WD], f16, kind="ExternalOutput").ap()

    with tile.TileContext(nc) as tc:
        with tc.tile_pool(name="at", bufs=1) as atp, \
             tc.tile_pool(name="hb", bufs=1) as hbp_pool, \
             tc.tile_pool(name="wet", bufs=6) as wetp, \
             tc.tile_pool(name="sl", bufs=6) as selp, \
             tc.tile_pool(name="oc", bufs=6) as ocp, \
             tc.tile_pool(name="pb", bufs=4) as pbp, \
             tc.tile_pool(name="ob", bufs=5) as obp, \
             tc.tile_pool(name="ps_pre", bufs=2, space="PSUM") as pspre, \
             tc.tile_pool(name="ps_exp", bufs=6, space="PSUM") as psexp:
            # chunk 0 weights first so the PE can start ASAP; AT k-slices
            # stream in the order the first k-chain consumes them.
            wb0 = wetp.tile([128, WD], f16, tag="wet")
            nc.sync.dma_start(wb0[:], weTb_ap[0])
            at = atp.tile([128, 8 * WD], f16)
            for k in range(8):
                nc.sync.dma_start(at[:, k * WD:(k + 1) * WD], ATb_ap[k])
            hb = hbp_pool.tile([NE, WD], f16)
            nc.sync.dma_start(hb[:], HBp_ap[:])

            def p_gemm(wb):
                if N1024:
                    pp = [pspre.tile([128, WD], f32, space="PSUM", tag="pp", name="pp")]
                    for k in range(8):
                        nc.tensor.matmul(pp[0][:], wb[:, k * 128:(k + 1) * 128],
                                         at[:, k * WD:(k + 1) * WD],
                                         start=(k == 0), stop=(k == 7))
                else:
                    pp = [pspre.tile([128, 512], f32, space="PSUM", tag="pp", name="pp")
                          for _ in range(2)]
                    for n in range(2):
                        for k in range(8):
                            nc.tensor.matmul(
                                pp[n][:], wb[:, k * 128:(k + 1) * 128],
                                at[:, k * WD + n * 512: k * WD + (n + 1) * 512],
                                start=(k == 0), stop=(k == 7))
                return pp

            def p_copy(pp, P):
                # PSUM -> SBUF fp16, halves split across DVE and ACT
                if N1024:
                    nc.vector.tensor_copy(P[:, 0:512], pp[0][:, 0:512])
                    nc.scalar.copy(P[:, 512:1024], pp[0][:, 512:1024])
                else:
                    nc.vector.tensor_copy(P[:, 0:512], pp[0][:])
                    nc.scalar.copy(P[:, 512:1024], pp[1][:])

            # prologue: chunk 0 P-GEMM
            pp_cur = p_gemm(wb0)

            tg = 0
            for c in range(nch):
                tpc = int(tiles_per_chunk[c])
                if c + 1 < nch:
                    wbn = wetp.tile([128, WD], f16, tag="wet")
                    nc.sync.dma_start(wbn[:], weTb_ap[c + 1])
                cs = tg * 128
                sl = selp.tile([128, tpc * 128], fp8, tag="sl")
                nc.sync.dma_start(sl[:], sel_ap[:, cs:cs + tpc * 128])
                oc = ocp.tile([NE, tpc * 128], fp8, tag="oc")
                nc.sync.dma_start(oc[:], ocid_ap[:, cs:cs + tpc * 128])

                P = pbp.tile([128, WD], f16, tag="pb")
                p_copy(pp_cur, P)
                if c + 1 < nch:
                    pp_cur = p_gemm(wbn)

                ob = obp.tile([128, tpc * WD], f16, tag="ob")
                # batch expansion in groups of 3 tiles: 6 sel matmuls then 6 oc
                # matmuls -- uninterrupted PE runs (measured 216ns/MM vs 320ns
                # for per-bank sel/oc interleave) within 6 PSUM banks.
                for i0 in range(0, tpc, 3):
                    grp = range(i0, min(i0 + 3, tpc))
                    pes = {}
                    for t in grp:
                        for n in range(2):
                            pe = psexp.tile([128, 512], f32, space="PSUM",
                                            tag="pe", name="pe")
                            pes[t, n] = pe
                            nc.tensor.matmul(pe[:], sl[:, t * 128:(t + 1) * 128],
                                             P[:, n * 512:(n + 1) * 512],
                                             start=True, stop=False)
                    for t in grp:
                        for n in range(2):
                            pe = pes[t, n]
                            nc.tensor.matmul(pe[:], oc[:, t * 128:(t + 1) * 128],
                                             hb[:, n * 512:(n + 1) * 512],
                                             start=False, stop=True)
                            dst = ob[:, t * WD + n * 512: t * WD + (n + 1) * 512]
                            if n == 0:
                                nc.scalar.activation(
                                    dst, pe[:], mybir.ActivationFunctionType.Relu)
                            else:
                                nc.vector.tensor_scalar_max(dst, pe[:], 0.0)
                tg += tpc
                # one merged store for the chunk's tiles
                if c < nch - 1:
                    dram = out_ap[(tg - tpc) * 128: tg * 128, :].rearrange(
                        "(i p) f -> p i f", p=128)
                    nc.gpsimd.dma_start(dram, ob[:].rearrange("p (i f) -> p i f", f=WD))
                else:
                    # last chunk: store per tile so the final rows drain early
                    for i in range(tpc):
                        t0 = (tg - tpc + i) * 128
                        nc.gpsimd.dma_start(
                            out_ap[t0:t0 + 128, :],
                            ob[:, i * WD:(i + 1) * WD])
    nc.compile()
    return nc


def kernel(word_emb, char_ids, word_ids, E, W_ih, b_ih, b_hh, W_lin, b_lin,
           _timing=None, _trace_cores=None, _sim_core=None):
    word_emb = np.asarray(word_emb, np.float32)
    char_ids = np.asarray(char_ids, np.int32)
    word_ids = np.asarray(word_ids, np.int32)
    E = np.asarray(E, np.float32)
    W_ih = np.asarray(W_ih, np.float32)
    b_ih = np.asarray(b_ih, np.float32)
    b_hh = np.asarray(b_hh, np.float32)
    W_lin = np.asarray(W_lin, np.float32)
    b_lin = np.asarray(b_lin, np.float32)

    T = char_ids.shape[0]
    NW = word_emb.shape[0]

    HBp = _hb_table(E, W_ih, b_ih, b_hh, W_lin, b_lin)
    A = np.ascontiguousarray(W_lin[:, :WD])

    word_bin, word_slot, nbins, bin_chars, bin_words = _pack_bins(word_ids, NW)

    # deal bins to cores by descending char count: rank r -> core r % NCORES,
    # chunk r // NCORES; the rank ordering makes chunk char counts uniform
    # across cores so the shared tiles_per_chunk wastes little padding.
    rank_of_bin = np.empty(nbins, np.int64)
    rank_of_bin[np.argsort(-bin_chars, kind="stable")] = np.arange(nbins)
    core_of_bin = (rank_of_bin % NCORES).astype(np.int32)
    chunk_of_bin = (rank_of_bin // NCORES).astype(np.int32)
    nch = nbins // NCORES

    # chars sorted by (bin, slot) -> contiguous per bin, word-major inside
    cb = word_bin[word_ids]
    cslot = word_slot[word_ids]
    ckey = cb.astype(np.int64) * 1024 + cslot
    corder = np.argsort(ckey, kind="stable")
    per_bin = np.bincount(cb, minlength=nbins)
    bstart = np.concatenate([[0], np.cumsum(per_bin)])

    chunk_cnt = np.zeros((NCORES, nch), np.int64)
    for b in range(nbins):
        chunk_cnt[core_of_bin[b], chunk_of_bin[b]] = per_bin[b]
    while nch > 1 and chunk_cnt[:, nch - 1].max() == 0:
        nch -= 1
    tiles_per_chunk = np.maximum(1, np.ceil(chunk_cnt[:, :nch].max(axis=0) / 128).astype(np.int64))
    ntiles = int(tiles_per_chunk.sum())
    tile_base = np.concatenate([[0], np.cumsum(tiles_per_chunk)])

    ATb = np.ascontiguousarray(A.T.reshape(8, 128, WD)).astype(NP16)
    HBq = HBp.astype(NP16)
    in_maps = []
    origs = []
    for m in range(NCORES):
        weTb = np.zeros((nch, 128, WD), NP16)
        sel = np.zeros((128, ntiles * 128), NP8)
        ocid = np.zeros((NE, ntiles * 128), NP8)
        orig = np.full(ntiles * 128, -1, np.int64)
        for c in range(nch):
            bs = np.nonzero((core_of_bin == m) & (chunk_of_bin == c))[0]
            if len(bs) == 0:
                continue
            b = bs[0]
            lo, hi = bstart[b], bstart[b + 1]
            chars = corder[lo:hi]
            wlist = np.nonzero(word_bin == b)[0]
            wlist = wlist[np.argsort(word_slot[wlist])]
            nwb = len(wlist)
            if nwb:
                rows = word_emb[wlist]  # [nwb, WD]
                blk = rows.T.reshape(8, 128, nwb).transpose(1, 0, 2)
                weTb[c, :, :].reshape(128, 8, 128)[:, :, :nwb] = blk
            q = np.arange(len(chars))
            col = tile_base[c] * 128 + q
            sel[cslot[chars], col] = 1.0
            ocid[char_ids[chars], col] = 1.0
            orig[col] = chars
        in_maps.append({
            "weTb": weTb,
            "ATb": ATb,
            "HBp": HBq,
            "sel": sel,
            "ocid": ocid,
        })
        origs.append(orig)

    nc = _build_program(nch, tiles_per_chunk)

    if _sim_core is not None:
        from concourse.bass_interp import CoreSim
        sim = CoreSim(nc, trace=False)
        for k, v in in_maps[_sim_core].items():
            sim.tensor(k)[:] = v
        sim.simulate(check_with_hw=False)
        o = np.asarray(sim.tensor("out"), np.float32)
        out = np.full((T, WD), np.nan, np.float32)
        v = origs[_sim_core] >= 0
        out[origs[_sim_core][v]] = o[v]
        return out

    kwargs = {}
    if _trace_cores is not None:
        kwargs = dict(trace=True, trace_cores=_trace_cores)
    res = run_bass_kernel_spmd(nc, in_maps, core_ids=list(range(NCORES)), **kwargs)
    if _timing is not None:
        _timing["exec_time_ns"] = res.exec_time_ns
        _timing["results"] = res

    out = np.empty((T, WD), np.float32)
    for m in range(NCORES):
        o = np.asarray(res.results[m]["out"], np.float32)
        v = origs[m] >= 0
        out[origs[m][v]] = o[v]
    return out


# revision 9
# speedup vs baseline: 1.1774x; 1.1774x over previous
"""CharLevelEncoder Trainium2 kernel (8-core SPMD).

Math: out = relu(concat(word_emb[word_ids], h(char_ids)) @ W_lin.T + b_lin)
with h a single LSTM cell step from zero state on E[char_ids].

Key algebraic restructuring:
  - h depends only on char_id (40 values) -> HB table [40, WD]:
        HB[c] = h_c @ W_lin[:, WD:].T + b_lin
  - word_emb[word_ids] @ W_lin[:, :WD].T == (word_emb @ A.T)[word_ids]
    (A = W_lin[:, :WD]); P computed per used word, sharded by word bins.
  - out[t] = relu(P[word_ids[t]] + HB[char_ids[t]])

Implementation notes (v2, measured-cost-model driven):
  - matmul cost on TRN2 ~= rhs-free-columns x 0.42ns (bf16) regardless
    of contraction rows; fp8 DoubleRow gives no net win for
    16-bit-precision data (2x8-bit slots/cycle == 1x16-bit) and fp16
    measures ~20% slower than bf16, so all matmul data is bf16.
  - one-hot operands (sel slot-gather, oc char-id HB-add) are fp8e4m3
    (exact for 0/1, halves their DMA).
  - output stored fp16 (halves the dominant DMA stream vs f32).
  - software pipeline: iter c runs P-copy(c) on ACT/DVE, P-GEMM(c+1) on
    PE, then expansion(c) on PE, so the PE never waits on the copies.
"""

import os

import ml_dtypes
import numpy as np

import concourse.bass as bass
import concourse.tile as tile
from concourse import bacc, mybir
from concourse.bass_utils import run_bass_kernel_spmd

NCORES = 8
WD = 1024
NE = 40
HID = 512
BIN_CHAR_CAP = 512
BIN_WORD_CAP = 128

# N=1024 two-bank PSUM matmuls are rejected by hardware (PSUM bank
# boundary), so N=512 is the only live path.
N1024 = os.environ.get("KERNEL_N1024", "0") == "1"

NPBF = ml_dtypes.bfloat16
NP16 = np.float16
NP8 = ml_dtypes.float8_e4m3


def _sigmoid(x):
    return 1.0 / (1.0 + np.exp(-x))


def _hb_table(E, W_ih, b_ih, b_hh, W_lin, b_lin):
    G = E.astype(np.float32) @ W_ih.T + b_ih + b_hh  # [NE, 4H]
    i, f, g, o = np.split(G, 4, axis=1)
    c = _sigmoid(i) * np.tanh(g)
    h = _sigmoid(o) * np.tanh(c)  # [NE, H]
    return (h @ W_lin[:, WD:].T + b_lin).astype(np.float32)  # [NE, WD]


def _pack_bins(word_ids, n_words):
    """Pack words into bins of <=BIN_WORD_CAP words and <=BIN_CHAR_CAP chars.
    Snake-deal of words sorted by char count keeps bin char totals equal."""
    wc = np.bincount(word_ids, minlength=n_words)
    used = np.nonzero(wc)[0]
    counts = wc[used]
    assert counts.max() <= BIN_CHAR_CAP, "single word exceeds bin capacity"
    order = np.argsort(-counts, kind="stable")
    wsorted = used[order]
    csorted = counts[order]
    nused = len(used)
    total = int(counts.sum())

    nbins = max(int(np.ceil(nused / (BIN_WORD_CAP - 2))), int(np.ceil(total / (BIN_CHAR_CAP - 12))))
    nbins = ((nbins + NCORES - 1) // NCORES) * NCORES
    while True:
        bin_of = np.empty(nused, np.int32)
        for r in range(int(np.ceil(nused / nbins))):
            lo, hi = r * nbins, min((r + 1) * nbins, nused)
            idx = np.arange(lo, hi)
            if r % 2 == 0:
                bin_of[idx] = idx - lo
            else:
                bin_of[idx] = nbins - 1 - (idx - lo)
        sums = np.bincount(bin_of, weights=csorted, minlength=nbins)
        nword = np.bincount(bin_of, minlength=nbins)
        if sums.max() <= BIN_CHAR_CAP and nword.max() <= BIN_WORD_CAP:
            break
        nbins += NCORES
    # Repair pass: concentrate the slack into the NCORES*4 smallest bins so
    # most chunks fill whole 128-char tiles (fewer tiles => less padding).
    import bisect
    ndon = min(4 * NCORES, nbins)
    order_bins = np.argsort(-sums, kind="stable")
    donors = set(order_bins[nbins - ndon:].tolist())
    pool = sorted((int(csorted[j]), j) for j in range(nused)
                  if int(bin_of[j]) in donors)  # asc by count
    keys = [c for c, _ in pool]
    for b in order_bins[:nbins - ndon]:
        deficit = int(BIN_CHAR_CAP - sums[b])
        while deficit > 0 and nword[b] < BIN_WORD_CAP and pool:
            k = bisect.bisect_right(keys, deficit) - 1
            if k < 0:
                break
            c, j = pool.pop(k)
            keys.pop(k)
            src = int(bin_of[j])
            bin_of[j] = b
            sums[b] += c
            sums[src] -= c
            nword[b] += 1
            nword[src] -= 1
            deficit -= c

    word_bin = np.full(n_words, -1, np.int32)
    word_bin[wsorted] = bin_of
    word_slot = np.full(n_words, -1, np.int32)
    ord2 = np.argsort(bin_of, kind="stable")
    slots = np.arange(nused) - np.concatenate([[0], np.cumsum(np.bincount(bin_of, minlength=nbins))])[bin_of[ord2]]
    word_slot[wsorted[ord2]] = slots
    return word_bin, word_slot, nbins, sums, nword


def _build_program(nch, tiles_per_chunk):
    ntiles = int(np.sum(tiles_per_chunk))
    f32 = mybir.dt.float32
    f16 = mybir.dt.float16
    bf16 = mybir.dt.bfloat16
    fp8 = mybir.dt.float8e4
    nc = bacc.Bacc("TRN2", target_bir_lowering=False, debug=False, num_devices=NCORES)
    weTb_ap = nc.dram_tensor("weTb", [nch, 128, WD], bf16, kind="ExternalInput").ap()
    ATb_ap = nc.dram_tensor("ATb", [8, 128, WD], bf16, kind="ExternalInput").ap()
    HBp_ap = nc.dram_tensor("HBp", [NE, WD], bf16, kind="ExternalInput").ap()
    sel_ap = nc.dram_tensor("sel", [128, ntiles * 128], fp8, kind="ExternalInput").ap()
    ocid_ap = nc.dram_tensor("ocid", [NE, ntiles * 128], fp8, kind="ExternalInput").ap()
    out_ap = nc.dram_tensor("out", [ntiles * 128, WD], f16, kind="ExternalOutput").ap()

    with tile.TileContext(nc) as tc:
        with tc.tile_pool(name="at", bufs=1) as atp, \
             tc.tile_pool(name="hb", bufs=1) as hbp_pool, \
             tc.tile_pool(name="wet", bufs=6) as wetp, \
             tc.tile_pool(name="sl", bufs=6) as selp, \
             tc.tile_pool(name="oc", bufs=6) as ocp, \
             tc.tile_pool(name="pb", bufs=4) as pbp, \
             tc.tile_pool(name="ob", bufs=5) as obp, \
             tc.tile_pool(name="ps_pre", bufs=2, space="PSUM") as pspre, \
             tc.tile_pool(name="ps_exp", bufs=6, space="PSUM") as psexp:
            # chunk 0 weights first so the PE can start ASAP; AT k-slices
            # stream in the order the first k-chain consumes them.
            wb0 = wetp.tile([128, WD], bf16, tag="wet")
            nc.sync.dma_start(wb0[:], weTb_ap[0])
            at = atp.tile([128, 8 * WD], bf16)
            for k in range(8):
                nc.sync.dma_start(at[:, k * WD:(k + 1) * WD], ATb_ap[k])
            hb = hbp_pool.tile([NE, WD], bf16)
            nc.sync.dma_start(hb[:], HBp_ap[:])

            def p_gemm(wb):
                if N1024:
                    pp = [pspre.tile([128, WD], f32, space="PSUM", tag="pp", name="pp")]
                    for k in range(8):
                        nc.tensor.matmul(pp[0][:], wb[:, k * 128:(k + 1) * 128],
                                         at[:, k * WD:(k + 1) * WD],
                                         start=(k == 0), stop=(k == 7))
                else:
                    pp = [pspre.tile([128, 512], f32, space="PSUM", tag="pp", name="pp")
                          for _ in range(2)]
                    for n in range(2):
                        for k in range(8):
                            nc.tensor.matmul(
                                pp[n][:], wb[:, k * 128:(k + 1) * 128],
                                at[:, k * WD + n * 512: k * WD + (n + 1) * 512],
                                start=(k == 0), stop=(k == 7))
                return pp

            def p_copy(pp, P):
                # PSUM -> SBUF fp16, halves split across DVE and ACT
                if N1024:
                    nc.vector.tensor_copy(P[:, 0:512], pp[0][:, 0:512])
                    nc.scalar.copy(P[:, 512:1024], pp[0][:, 512:1024])
                else:
                    nc.vector.tensor_copy(P[:, 0:512], pp[0][:])
                    nc.scalar.copy(P[:, 512:1024], pp[1][:])

            # prologue: chunk 0 P-GEMM
            pp_cur = p_gemm(wb0)

            tg = 0
            for c in range(nch):
                tpc = int(tiles_per_chunk[c])
                if c + 1 < nch:
                    wbn = wetp.tile([128, WD], bf16, tag="wet")
                    nc.sync.dma_start(wbn[:], weTb_ap[c + 1])
                cs = tg * 128
                sl = selp.tile([128, tpc * 128], fp8, tag="sl")
                nc.sync.dma_start(sl[:], sel_ap[:, cs:cs + tpc * 128])
                oc = ocp.tile([NE, tpc * 128], fp8, tag="oc")
                nc.sync.dma_start(oc[:], ocid_ap[:, cs:cs + tpc * 128])

                P = pbp.tile([128, WD], bf16, tag="pb")
                p_copy(pp_cur, P)
                if c + 1 < nch:
                    pp_cur = p_gemm(wbn)

                ob = obp.tile([128, tpc * WD], f16, tag="ob")
                # batch expansion in groups of 3 tiles: 6 sel matmuls then 6 oc
                # matmuls -- uninterrupted PE runs (measured 216ns/MM vs 320ns
                # for per-bank sel/oc interleave) within 6 PSUM banks.
                for i0 in range(0, tpc, 3):
                    grp = range(i0, min(i0 + 3, tpc))
                    pes = {}
                    for t in grp:
                        for n in range(2):
                            pe = psexp.tile([128, 512], f32, space="PSUM",
                                            tag="pe", name="pe")
                            pes[t, n] = pe
                            nc.tensor.matmul(pe[:], sl[:, t * 128:(t + 1) * 128],
                                             P[:, n * 512:(n + 1) * 512],
                                             start=True, stop=False)
                    for t in grp:
                        for n in range(2):
                            pe = pes[t, n]
                            nc.tensor.matmul(pe[:], oc[:, t * 128:(t + 1) * 128],
                                             hb[:, n * 512:(n + 1) * 512],
                                             start=False, stop=True)
                            dst = ob[:, t * WD + n * 512: t * WD + (n + 1) * 512]
                            if n == 0:
                                nc.scalar.activation(
                                    dst, pe[:], mybir.ActivationFunctionType.Relu)
                            else:
                                nc.vector.tensor_scalar_max(dst, pe[:], 0.0)
                tg += tpc
                # one merged store for the chunk's tiles
                if c < nch - 1:
                    dram = out_ap[(tg - tpc) * 128: tg * 128, :].rearrange(
                        "(i p) f -> p i f", p=128)
                    nc.gpsimd.dma_start(dram, ob[:].rearrange("p (i f) -> p i f", f=WD))
                else:
                    # last chunk: store per tile so the final rows drain early
                    for i in range(tpc):
                        t0 = (tg - tpc + i) * 128
                        nc.gpsimd.dma_start(
                            out_ap[t0:t0 + 128, :],
                            ob[:, i * WD:(i + 1) * WD])
    nc.compile()
    return nc


def kernel(word_emb, char_ids, word_ids, E, W_ih, b_ih, b_hh, W_lin, b_lin,
           _timing=None, _trace_cores=None, _sim_core=None):
    word_emb = np.asarray(word_emb, np.float32)
    char_ids = np.asarray(char_ids, np.int32)
    word_ids = np.asarray(word_ids, np.int32)
    E = np.asarray(E, np.float32)
    W_ih = np.asarray(W_ih, np.float32)
    b_ih = np.asarray(b_ih, np.float32)
    b_hh = np.asarray(b_hh, np.float32)
    W_lin = np.asarray(W_lin, np.float32)
    b_lin = np.asarray(b_lin, np.float32)

    T = char_ids.shape[0]
    NW = word_emb.shape[0]

    HBp = _hb_table(E, W_ih, b_ih, b_hh, W_lin, b_lin)
    A = np.ascontiguousarray(W_lin[:, :WD])

    word_bin, word_slot, nbins, bin_chars, bin_words = _pack_bins(word_ids, NW)

    # deal bins to cores by descending char count: rank r -> core r % NCORES,
    # chunk r // NCORES; the rank ordering makes chunk char counts uniform
    # across cores so the shared tiles_per_chunk wastes little padding.
    rank_of_bin = np.empty(nbins, np.int64)
    rank_of_bin[np.argsort(-bin_chars, kind="stable")] = np.arange(nbins)
    core_of_bin = (rank_of_bin % NCORES).astype(np.int32)
    chunk_of_bin = (rank_of_bin // NCORES).astype(np.int32)
    nch = nbins // NCORES

    # chars sorted by (bin, slot) -> contiguous per bin, word-major inside
    cb = word_bin[word_ids]
    cslot = word_slot[word_ids]
    ckey = cb.astype(np.int64) * 1024 + cslot
    corder = np.argsort(ckey, kind="stable")
    per_bin = np.bincount(cb, minlength=nbins)
    bstart = np.concatenate([[0], np.cumsum(per_bin)])

    chunk_cnt = np.zeros((NCORES, nch), np.int64)
    for b in range(nbins):
        chunk_cnt[core_of_bin[b], chunk_of_bin[b]] = per_bin[b]
    while nch > 1 and chunk_cnt[:, nch - 1].max() == 0:
        nch -= 1
    tiles_per_chunk = np.maximum(1, np.ceil(chunk_cnt[:, :nch].max(axis=0) / 128).astype(np.int64))
    ntiles = int(tiles_per_chunk.sum())
    tile_base = np.concatenate([[0], np.cumsum(tiles_per_chunk)])

    ATb = np.ascontiguousarray(A.T.reshape(8, 128, WD)).astype(NPBF)
    HBq = HBp.astype(NPBF)
    in_maps = []
    origs = []
    for m in range(NCORES):
        weTb = np.zeros((nch, 128, WD), NPBF)
        sel = np.zeros((128, ntiles * 128), NP8)
        ocid = np.zeros((NE, ntiles * 128), NP8)
        orig = np.full(ntiles * 128, -1, np.int64)
        for c in range(nch):
            bs = np.nonzero((core_of_bin == m) & (chunk_of_bin == c))[0]
            if len(bs) == 0:
                continue
            b = bs[0]
            lo, hi = bstart[b], bstart[b + 1]
            chars = corder[lo:hi]
            wlist = np.nonzero(word_bin == b)[0]
            wlist = wlist[np.argsort(word_slot[wlist])]
            nwb = len(wlist)
            if nwb:
                rows = word_emb[wlist]  # [nwb, WD]
                blk = rows.T.reshape(8, 128, nwb).transpose(1, 0, 2)
                weTb[c, :, :].reshape(128, 8, 128)[:, :, :nwb] = blk
            q = np.arange(len(chars))
            col = tile_base[c] * 128 + q
            sel[cslot[chars], col] = 1.0
            ocid[char_ids[chars], col] = 1.0
            orig[col] = chars
        in_maps.append({
            "weTb": weTb,
            "ATb": ATb,
            "HBp": HBq,
            "sel": sel,
            "ocid": ocid,
        })
        origs.append(orig)

    nc = _build_program(nch, tiles_per_chunk)

    if _sim_core is not None:
        from concourse.bass_interp import CoreSim
        sim = CoreSim(nc, trace=False)
        for k, v in in_maps[_sim_core].items():
            sim.tensor(k)[:] = v
        sim.simulate(check_with_hw=False)
        o = np.asarray(sim.tensor("out"), np.float32)
        out = np.full((T, WD), np.nan, np.float32)
        v = origs[_sim_core] >= 0
        out[origs[_sim_core][v]] = o[v]
        return out

    kwargs = {}
    if _trace_cores is not None:
        kwargs = dict(trace=True, trace_cores=_trace_cores)
    res = run_bass_kernel_spmd(nc, in_maps, core_ids=list(range(NCORES)), **kwargs)
    if _timing is not None:
        _timing["exec_time_ns"] = res.exec_time_ns
        _timing["results"] = res

    out = np.empty((T, WD), np.float32)
    for m in range(NCORES):
        o = np.asarray(res.results[m]["out"], np.float32)
        v = origs[m] >= 0
        out[origs[m][v]] = o[v]
    return out
